# revision 4
# baseline (speedup 1.0000x reference)
# DiGCN Inception Block (2 blocks, 3 branches each) on 8 TRN2 NeuronCores.
#
# Math per block:  out = x @ ln_w + A1 @ c1_w + A2 @ c2_w + (b_ln + b_c1 + b_c2)
# where Ak = segsum_dst(ewk * x[src])  (matmul commutes with the weighted
# segment-sum, so raw node rows are aggregated and the weight matmul applied
# after).
#
# Sharding (edge partitioning by dst, standard graph data parallelism):
# nodes are assigned to (core, tile-of-64, lane) with in-degree balancing so
# every 64-node tile owns ~E/(M*TILES) incoming edges.  At shard time the host
# replicates the remote src-node embedding rows into a per-edge message table
# laid out exactly as the device consumes it (tile, subtile-of-128, lane) --
# "replicate remote src node embeddings" -- so the device reads one fully
# sequential fp16 stream per tile instead of doing a random dma_gather per
# edge.  All arithmetic of the reference (ew scaling, segment sums, weight
# matmuls, bias adds) happens on device:
#   S   = one-hot(dst_rel)            (DVE is_equal, fp16, 2x mode)
#   S1  = ew1*S ; S2 = ew2*S          (DVE mult)
#   Ak^T[ch,n] += msg_j^T @ Sk_j      (PE, per 128-edge subtile, PSUM accum)
#   out^T = w^T [A1s|A2s|own] + bias  (PE + ACT)
# Outputs are written transposed ([d_out, nodes]); the host undoes the node
# permutation.  Between the two blocks the host only re-lays-out block1's
# output (cast fp16, replicate rows per edge) -- no host arithmetic.

import os
import sys

for _p in ("/opt/trn_rl_repo", "/root/.axon_site/_ro/trn_rl_repo"):
    if os.path.isdir(_p) and _p not in sys.path:
        sys.path.insert(0, _p)
        break

import heapq

import numpy as np

import concourse.bacc as bacc
import concourse.tile as tile
import concourse.mybir as mybir
from concourse import bass_utils


class Cfg:
    def __init__(self, n, e, f_in, emb, out, m=8, pn=32, grp=28, tiles=None,
                 sb=4):
        self.N, self.E, self.F, self.EMB, self.OUT = n, e, f_in, emb, out
        self.M = m
        self.NPC = n // m                  # nodes per core
        self.PN = pn                       # nodes per tile
        self.TILES = tiles or -(-self.NPC // pn)   # tiles per core
        self.TILES = -(-self.TILES // grp) * grp   # round up to DMA groups
        self.SLOTS = self.TILES * pn       # node slots per core (>= NPC)
        self.GRP = grp                     # tiles per DMA group
        self.SB = sb                       # tiles per S-build / PSUM batch
        assert self.GRP % sb == 0
        self.MSG_ENGINES = ("sync", "scalar", "gpsimd") * 2  # msg DMA queues
        self.OUT_ENGINE = "sync"
        # Pool/GPSIMD does not support TensorTensor on real HW (walrus
        # "Instruction engine check failed (Pool)") — S-builds stay on DVE.
        self.POOL_S_MOD = 0     # every k-th S-build on Pool (0 = all on DVE)


# TILES=420 (not the minimal 392) so balanced tile loads stay under
# NT=4 subtiles (mean in-degree per tile 476 <= 512) with margin.
FULL = Cfg(100000, 1600000, 128, 64, 32, pn=32, grp=32, tiles=416, sb=8)


# ---------------------------------------------------------------- host prep

def _greedy_assign(weights, n_bins, cap):
    """LPT: heavy items first to the least-loaded non-full bin.
    Returns bin index per item and the max bin load."""
    order = np.argsort(-weights, kind="stable")
    out = np.empty(len(weights), np.int32)
    loads = [0.0] * n_bins
    counts = [0] * n_bins
    heap = [(0.0, b) for b in range(n_bins)]
    heapq.heapify(heap)
    for it in order:
        while True:
            load, b = heapq.heappop(heap)
            if counts[b] < cap:
                break
        out[it] = b
        counts[b] += 1
        loads[b] = load + weights[it]
        if counts[b] < cap:
            heapq.heappush(heap, (loads[b], b))
    return out, max(loads)


def _plan(cfg, src, dst):
    """Node->(core,tile,lane) balanced by in-degree; edge->slot layout.
    Returns dict with the permutation and slot tables."""
    N, M, T, PN = cfg.N, cfg.M, cfg.TILES, cfg.PN
    src = np.ascontiguousarray(src, np.int64).ravel()
    dst = np.ascontiguousarray(dst, np.int64).ravel()
    deg = np.bincount(dst, minlength=N).astype(np.float64)

    core_of, _ = _greedy_assign(deg, M, cfg.NPC)
    tile_of = np.empty(N, np.int32)
    lane_of = np.empty(N, np.int32)
    new2old = np.full((M, T * PN), -1, np.int64)
    maxload = 0
    for m in range(M):
        idx = np.nonzero(core_of == m)[0]
        tl, ml = _greedy_assign(deg[idx], T, PN)
        maxload = max(maxload, ml)
        tile_of[idx] = tl
        # lanes: stable order within tile
        order = np.argsort(tl, kind="stable")
        sidx = idx[order]
        stl = tl[order]
        pos = np.arange(len(sidx)) - np.searchsorted(stl, stl)
        lane_of[sidx] = pos
        new2old[m, stl * PN + pos] = sidx

    NT = max(1, int(-(-maxload // 128)))

    # edge slots
    tg = (core_of[dst].astype(np.int64) * T + tile_of[dst])
    order = np.argsort(tg, kind="stable")
    counts = np.bincount(tg, minlength=M * T)
    assert counts.max() <= NT * 128
    starts = np.zeros(M * T + 1, np.int64)
    np.cumsum(counts, out=starts[1:])
    pos = np.arange(cfg.E, dtype=np.int64) - starts[tg[order]]
    j = pos // 128
    p = pos % 128

    slot_src = np.zeros((M, T, NT, 128), np.int64)
    slot_dstrel = np.full((M, T, NT, 128), -1.0, np.float16)
    slot_ew1 = np.zeros((M, T, NT, 128), np.float16)
    slot_ew2 = np.zeros((M, T, NT, 128), np.float16)
    tgo = tg[order]
    m_ = tgo // T
    t_ = tgo % T
    eo = order
    slot_src[m_, t_, j, p] = src[eo]
    slot_dstrel[m_, t_, j, p] = lane_of[dst[eo]].astype(np.float16)
    return dict(core_of=core_of, tile_of=tile_of, lane_of=lane_of,
                new2old=new2old, NT=NT, slot_src=slot_src,
                slot_dstrel=slot_dstrel, slot_idx=(m_, t_, j, p), eo=eo,
                s1=slot_ew1, s2=slot_ew2)


def _fill_ew(cfg, plan, ew1, ew2):
    m_, t_, j, p = plan["slot_idx"]
    eo = plan["eo"]
    plan["s1"][m_, t_, j, p] = np.asarray(ew1, np.float32).ravel()[eo]
    plan["s2"][m_, t_, j, p] = np.asarray(ew2, np.float32).ravel()[eo]


def _msg_table(cfg, plan, table16):
    """Per-edge replicated src rows: [M, 128, T*F*NT], col (t*F+ch)*NT+j."""
    M, T, NT = cfg.M, cfg.TILES, plan["NT"]
    d = table16.shape[1]
    g = table16[plan["slot_src"]]              # [M, T, NT, 128, d]
    g = np.ascontiguousarray(g.transpose(0, 3, 1, 4, 2))  # [M,128,T,d,NT]
    return g.reshape(M, 128, T * d * NT)


def _aux_table(cfg, plan):
    """[M, 128, T*3*NT], cols per tile: dst|ew1|ew2 each [NT]."""
    M, T, NT = cfg.M, cfg.TILES, plan["NT"]
    a = np.stack([plan["slot_dstrel"], plan["s1"], plan["s2"]], axis=2)
    # [M, T, 3, NT, 128] -> [M, 128, T, 3, NT]
    a = np.ascontiguousarray(a.transpose(0, 4, 1, 2, 3))
    return a.reshape(M, 128, T * 3 * NT)


def _own_table(cfg, plan, table16):
    """[M, d, T*PN]: column t*PN+n = row of own node (0 for pad lanes)."""
    M = cfg.M
    d = table16.shape[1]
    n2o = plan["new2old"]                      # [M, SLOTS]
    rows = table16[np.maximum(n2o, 0)]         # [M, SLOTS, d]
    rows[n2o < 0] = 0
    return np.ascontiguousarray(rows.transpose(0, 2, 1))


def _iota_const(cfg, NT):
    io = np.zeros((128, cfg.PN * NT), np.float16)
    io[:] = (np.arange(cfg.PN)[:, None] * np.ones((1, NT))).reshape(1, -1)
    return io


# ------------------------------------------------------------- device build

def _build_block(cfg, d_in, d_out, NT, out_f32):
    f16 = mybir.dt.float16
    f32 = mybir.dt.float32
    T, PN, GRP, SB = cfg.TILES, cfg.PN, cfg.GRP, cfg.SB
    GROUPS = T // GRP
    odt = f32 if out_f32 else f16

    nc = bacc.Bacc("TRN2", target_bir_lowering=False, debug=False,
                   num_devices=cfg.M)
    msg = nc.dram_tensor("msg", [128, T * d_in * NT], f16,
                         kind="ExternalInput")
    aux = nc.dram_tensor("aux", [128, T * 3 * NT], f16, kind="ExternalInput")
    own = nc.dram_tensor("own", [d_in, T * PN], f16, kind="ExternalInput")
    wts = nc.dram_tensor("wts", [d_in, 3 * d_out], f16, kind="ExternalInput")
    iot = nc.dram_tensor("iot", [128, PN * NT], f16, kind="ExternalInput")
    bia = nc.dram_tensor("bia", [d_out, 1], f32, kind="ExternalInput")
    out = nc.dram_tensor("out", [d_out, T * PN], odt, kind="ExternalOutput")

    MW = d_in * NT      # msg cols per tile
    AW = 3 * NT         # aux cols per tile

    with tile.TileContext(nc) as tc:
        with (
            tc.tile_pool(name="const", bufs=1) as cpool,
            tc.tile_pool(name="io", bufs=2) as iop,
            tc.tile_pool(name="sb", bufs=3) as pool,
            tc.tile_pool(name="ps", bufs=3, space="PSUM") as psum,
            tc.tile_pool(name="ps2", bufs=2, space="PSUM") as psum2,
        ):
            wts_t = cpool.tile([d_in, 3 * d_out], f16, tag="wts")
            nc.sync.dma_start(out=wts_t[:], in_=wts[:, :])
            iot_t = cpool.tile([128, PN * NT], f16, tag="iot")
            nc.sync.dma_start(out=iot_t[:], in_=iot[:, :])
            bia_t = cpool.tile([d_out, 1], f32, tag="bia")
            nc.sync.dma_start(out=bia_t[:], in_=bia[:, :])

            def tail(prev):
                # second-stage matmuls + bias for an earlier quad of tiles;
                # emitted after the next quad's aggregation so PE doesn't
                # block on the ACT evacuation
                a12s, own_g, out_g, base = prev
                ypp = psum2.tile([d_out, SB * PN], f32, tag="y", space="PSUM")
                for tt in range(SB):
                    nc.tensor.matmul(
                        out=ypp[:, tt * PN:(tt + 1) * PN],
                        lhsT=wts_t[:, 0:d_out],
                        rhs=a12s[:, tt * 2 * PN: tt * 2 * PN + PN],
                        start=True, stop=False)
                    nc.tensor.matmul(
                        out=ypp[:, tt * PN:(tt + 1) * PN],
                        lhsT=wts_t[:, d_out:2 * d_out],
                        rhs=a12s[:, tt * 2 * PN + PN:(tt + 1) * 2 * PN],
                        start=False, stop=False)
                    nc.tensor.matmul(
                        out=ypp[:, tt * PN:(tt + 1) * PN],
                        lhsT=wts_t[:, 2 * d_out:3 * d_out],
                        rhs=own_g[:, (base + tt) * PN:(base + tt + 1) * PN],
                        start=False, stop=True)
                nc.scalar.activation(
                    out=out_g[:, base * PN:(base + SB) * PN], in_=ypp[:],
                    func=mybir.ActivationFunctionType.Identity,
                    bias=bia_t[:, 0:1])

            prev = None
            for g in range(GROUPS):
                msg_g = iop.tile([128, GRP * MW], f16, tag="msg")
                W = GRP * MW
                me = [getattr(nc, e) for e in cfg.MSG_ENGINES]
                cw = W // len(me)
                for k, eng in enumerate(me):
                    lo = k * cw
                    hi = (k + 1) * cw if k < len(me) - 1 else W
                    eng.dma_start(
                        out=msg_g[:, lo:hi],
                        in_=msg[:, g * W + lo: g * W + hi])
                aux_g = iop.tile([128, GRP * AW], f16, tag="aux")
                nc.gpsimd.dma_start(
                    out=aux_g[:], in_=aux[:, g * GRP * AW:(g + 1) * GRP * AW])
                own_g = iop.tile([d_in, GRP * PN], f16, tag="own")
                nc.gpsimd.dma_start(
                    out=own_g[:], in_=own[:, g * GRP * PN:(g + 1) * GRP * PN])
                out_g = iop.tile([d_out, GRP * PN], odt, tag="out")

                for q in range(GRP // SB):
                    base = q * SB
                    # every k-th S-build runs on the Pool engine to unload DVE
                    ksm = cfg.POOL_S_MOD
                    veng = nc.gpsimd if ksm and (
                        g * (GRP // SB) + q) % ksm == ksm - 1 else nc.vector
                    # S matrices for 4 tiles in 3 DVE ops (fp16, 2x mode)
                    s_q = pool.tile([128, SB * PN * NT], f16, tag="s")
                    s12q = pool.tile([128, SB * 2 * PN * NT], f16, tag="s12")
                    sqr = s_q[:].rearrange("p (t n j) -> p t n j", t=SB, j=NT)
                    s12r = s12q[:].rearrange("p (t n j) -> p t n j",
                                             t=SB, j=NT)
                    aux4 = aux_g[:, base * AW:(base + SB) * AW].rearrange(
                        "p (t k j) -> p t k j", t=SB, j=NT)
                    iob = iot_t[:].rearrange(
                        "p (n j) -> p n j", j=NT).unsqueeze(1).to_broadcast(
                        [128, SB, PN, NT])
                    veng.tensor_tensor(
                        out=sqr,
                        in0=aux4[:, :, 0].unsqueeze(2).to_broadcast(
                            [128, SB, PN, NT]),
                        in1=iob, op=mybir.AluOpType.is_equal)
                    veng.tensor_tensor(
                        out=s12r[:, :, 0:PN], in0=sqr,
                        in1=aux4[:, :, 1].unsqueeze(2).to_broadcast(
                            [128, SB, PN, NT]),
                        op=mybir.AluOpType.mult)
                    veng.tensor_tensor(
                        out=s12r[:, :, PN:2 * PN], in0=sqr,
                        in1=aux4[:, :, 2].unsqueeze(2).to_broadcast(
                            [128, SB, PN, NT]),
                        op=mybir.AluOpType.mult)

                    # aggregation: one matmul per (tile, subtile), both
                    # branches via rhs = [S1|S2]; 4 tiles share one PSUM bank
                    a12p = psum.tile([d_in, SB * 2 * PN], f32, tag="a12",
                                     space="PSUM")
                    for tt in range(SB):
                        msg3 = msg_g[:, (base + tt) * MW:
                                     (base + tt + 1) * MW].rearrange(
                            "p (c j) -> p c j", j=NT)
                        for jj in range(NT):
                            nc.tensor.matmul(
                                out=a12p[:, tt * 2 * PN:(tt + 1) * 2 * PN],
                                lhsT=msg3[:, :, jj],
                                rhs=s12r[:, tt, :, jj],
                                start=(jj == 0), stop=(jj == NT - 1))
                    if prev is not None:
                        tail(prev)
                    a12s = pool.tile([d_in, SB * 2 * PN], f16, tag="a12s")
                    nc.scalar.activation(
                        out=a12s[:], in_=a12p[:],
                        func=mybir.ActivationFunctionType.Copy)
                    prev = (a12s, own_g, out_g, base)
                    if q == GRP // SB - 1:
                        tail(prev)      # flush before out_g is stored
                        prev = None

                getattr(nc, cfg.OUT_ENGINE).dma_start(
                    out=out[:, g * GRP * PN:(g + 1) * GRP * PN],
                    in_=out_g[:])

    nc.compile()
    return nc


_BUILD_CACHE = {}


def _get_block(cfg, d_in, d_out, NT, out_f32):
    key = (cfg.N, cfg.E, cfg.PN, cfg.GRP, cfg.TILES, d_in, d_out, NT, out_f32)
    if key not in _BUILD_CACHE:
        _BUILD_CACHE[key] = _build_block(cfg, d_in, d_out, NT, out_f32)
    return _BUILD_CACHE[key]


def _run_block(cfg, ncb, msg, aux, own, wts, iot, bia):
    in_maps = []
    for c in range(cfg.M):
        in_maps.append({
            "msg": msg[c], "aux": aux[c], "own": own[c],
            "wts": wts, "iot": iot, "bia": bia,
        })
    res = bass_utils.run_bass_kernel_spmd(
        ncb, in_maps, core_ids=list(range(cfg.M)))
    return np.stack([r["out"] for r in res.results])   # [M, d_out, SLOTS]


def _prep_plan(cfg, src, dst, ew1, ew2):
    plan = _plan(cfg, src, dst)
    _fill_ew(cfg, plan, ew1, ew2)
    plan["aux"] = _aux_table(cfg, plan)
    plan["iot"] = _iota_const(cfg, plan["NT"])
    return plan


def _wts16(w1, w2, wl, b1, b2, bl):
    w = np.ascontiguousarray(np.concatenate(
        [w1, w2, wl], axis=1), np.float32).astype(np.float16)
    b = np.ascontiguousarray(
        (np.asarray(bl) + np.asarray(b1) + np.asarray(b2))
        .reshape(-1, 1), np.float32)
    return w, b


def _prep_block(cfg, plan, table16, d_out, wts, bia, own_t=None):
    """(nc, in_maps-style arrays) for one inception block."""
    d_in = table16.shape[1]
    msg = _msg_table(cfg, plan, table16)
    own = own_t if own_t is not None else _own_table(cfg, plan, table16)
    nc = _get_block(cfg, d_in, d_out, plan["NT"], out_f32=False)
    return nc, (msg, plan["aux"], own, wts, plan["iot"], bia)


def _unperm(cfg, plan, rT, dtype):
    """[M, d, SLOTS] device output -> [N, d] in original node order."""
    full = np.zeros((cfg.N, rT.shape[1]), dtype)
    for m in range(cfg.M):
        n2o = plan["new2old"][m]
        real = n2o >= 0
        full[n2o[real]] = rT[m][:, real].T
    return full


def _kernel_cfg(cfg, features, ew1, ew2, src, dst,
                ln1_w, ln1_b, c11_w, c11_b, c12_w, c12_b,
                ln2_w, ln2_b, c21_w, c21_b, c22_w, c22_b):
    plan = _prep_plan(cfg, src, dst, ew1, ew2)
    feat16 = np.ascontiguousarray(features, np.float32).astype(np.float16)
    wts1, bia1 = _wts16(c11_w, c12_w, ln1_w, c11_b, c12_b, ln1_b)
    nc1, args1 = _prep_block(cfg, plan, feat16, cfg.EMB, wts1, bia1)
    xT = _run_block(cfg, nc1, *args1)
    x16 = _unperm(cfg, plan, xT, np.float16)

    wts2, bia2 = _wts16(c21_w, c22_w, ln2_w, c21_b, c22_b, ln2_b)
    nc2, args2 = _prep_block(cfg, plan, x16, cfg.OUT, wts2, bia2,
                             own_t=np.ascontiguousarray(xT))
    yT = _run_block(cfg, nc2, *args2)
    return _unperm(cfg, plan, yT, np.float32).astype(np.float32)


def kernel(features, ew1, ew2, src, dst,
           ln1_w, ln1_b, c11_w, c11_b, c12_w, c12_b,
           ln2_w, ln2_b, c21_w, c21_b, c22_w, c22_b):
    return _kernel_cfg(FULL, features, ew1, ew2, src, dst,
                       ln1_w, ln1_b, c11_w, c11_b, c12_w, c12_b,
                       ln2_w, ln2_b, c21_w, c21_b, c22_w, c22_b)


# revision 5
# speedup vs baseline: 1.3844x; 1.3844x over previous
# DiGCN Inception Block (2 blocks, 3 branches each) on 8 TRN2 NeuronCores.
#
# Math per block:  out = x @ ln_w + A1 @ c1_w + A2 @ c2_w + (b_ln + b_c1 + b_c2)
# where Ak = segsum_dst(ewk * x[src])  (matmul commutes with the weighted
# segment-sum, so raw node rows are aggregated and the weight matmul applied
# after).
#
# Sharding (edge partitioning by dst, standard graph data parallelism):
# nodes are assigned to (core, tile-of-PN, lane) with in-degree balancing so
# every PN-node tile owns ~E/(M*TILES) incoming edges (NT subtiles of 128).  At shard time the host
# replicates the remote src-node embedding rows into a per-edge message table
# laid out exactly as the device consumes it (tile, subtile-of-128, lane) --
# "replicate remote src node embeddings" -- so the device reads one fully
# sequential fp16 stream per tile instead of doing a random dma_gather per
# edge.  All arithmetic of the reference (ew scaling, segment sums, weight
# matmuls, bias adds) happens on device:
#   S   = one-hot(dst_rel)            (DVE is_equal, fp16, 2x mode)
#   S1  = ew1*S ; S2 = ew2*S          (DVE mult)
#   Ak^T[ch,n] += msg_j^T @ Sk_j      (PE, per 128-edge subtile, PSUM accum)
#   out^T = w^T [A1s|A2s|own] + bias  (PE + ACT)
# Outputs are written transposed ([d_out, nodes]); the host undoes the node
# permutation.  Between the two blocks the host only re-lays-out block1's
# output (cast fp16, replicate rows per edge) -- no host arithmetic.

import os
import sys

for _p in ("/opt/trn_rl_repo", "/root/.axon_site/_ro/trn_rl_repo"):
    if os.path.isdir(_p) and _p not in sys.path:
        sys.path.insert(0, _p)
        break

import heapq

import numpy as np

import concourse.bacc as bacc
import concourse.tile as tile
import concourse.mybir as mybir
from concourse import bass_utils


class Cfg:
    def __init__(self, n, e, f_in, emb, out, m=8, pn=32, grp=28, tiles=None,
                 sb=4):
        self.N, self.E, self.F, self.EMB, self.OUT = n, e, f_in, emb, out
        self.M = m
        self.NPC = n // m                  # nodes per core
        self.PN = pn                       # nodes per tile
        self.TILES = tiles or -(-self.NPC // pn)   # tiles per core
        self.TILES = -(-self.TILES // grp) * grp   # round up to DMA groups
        self.SLOTS = self.TILES * pn       # node slots per core (>= NPC)
        self.GRP = grp                     # tiles per DMA group
        self.SB = sb                       # tiles per S-build / PSUM batch
        assert self.GRP % sb == 0
        self.MSG_ENGINES = ("sync", "scalar", "gpsimd") * 2  # msg DMA queues
        self.OUT_ENGINE = "sync"
        # Pool/GPSIMD does not support TensorTensor on real HW (walrus
        # "Instruction engine check failed (Pool)") — S-builds stay on DVE.
        self.POOL_S_MOD = 0     # every k-th S-build on Pool (0 = all on DVE)


# TILES=416 (above the minimal 391) so balanced tile loads stay under
# NT=4 subtiles of 128 edges (mean in-degree per tile 481 <= 512).
FULL = Cfg(100000, 1600000, 128, 64, 32, pn=32, grp=32, tiles=416, sb=8)


# ---------------------------------------------------------------- host prep

def _greedy_assign(weights, n_bins, cap):
    """LPT: heavy items first to the least-loaded non-full bin.
    Returns bin index per item and the max bin load."""
    order = np.argsort(-weights, kind="stable")
    out = np.empty(len(weights), np.int32)
    loads = [0.0] * n_bins
    counts = [0] * n_bins
    heap = [(0.0, b) for b in range(n_bins)]
    heapq.heapify(heap)
    for it in order:
        while True:
            load, b = heapq.heappop(heap)
            if counts[b] < cap:
                break
        out[it] = b
        counts[b] += 1
        loads[b] = load + weights[it]
        if counts[b] < cap:
            heapq.heappush(heap, (loads[b], b))
    return out, max(loads)


def _plan(cfg, src, dst):
    """Node->(core,tile,lane) balanced by in-degree; edge->slot layout.
    Returns dict with the permutation and slot tables."""
    N, M, T, PN = cfg.N, cfg.M, cfg.TILES, cfg.PN
    src = np.ascontiguousarray(src, np.int64).ravel()
    dst = np.ascontiguousarray(dst, np.int64).ravel()
    deg = np.bincount(dst, minlength=N).astype(np.float64)

    core_of, _ = _greedy_assign(deg, M, cfg.NPC)
    tile_of = np.empty(N, np.int32)
    lane_of = np.empty(N, np.int32)
    new2old = np.full((M, T * PN), -1, np.int64)
    maxload = 0
    for m in range(M):
        idx = np.nonzero(core_of == m)[0]
        tl, ml = _greedy_assign(deg[idx], T, PN)
        maxload = max(maxload, ml)
        tile_of[idx] = tl
        # lanes: stable order within tile
        order = np.argsort(tl, kind="stable")
        sidx = idx[order]
        stl = tl[order]
        pos = np.arange(len(sidx)) - np.searchsorted(stl, stl)
        lane_of[sidx] = pos
        new2old[m, stl * PN + pos] = sidx

    NT = max(1, int(-(-maxload // 128)))

    # edge slots
    tg = (core_of[dst].astype(np.int64) * T + tile_of[dst])
    order = np.argsort(tg, kind="stable")
    counts = np.bincount(tg, minlength=M * T)
    assert counts.max() <= NT * 128
    starts = np.zeros(M * T + 1, np.int64)
    np.cumsum(counts, out=starts[1:])
    pos = np.arange(cfg.E, dtype=np.int64) - starts[tg[order]]
    j = pos // 128
    p = pos % 128

    slot_src = np.zeros((M, T, NT, 128), np.int64)
    slot_dstrel = np.full((M, T, NT, 128), -1.0, np.float16)
    slot_ew1 = np.zeros((M, T, NT, 128), np.float16)
    slot_ew2 = np.zeros((M, T, NT, 128), np.float16)
    tgo = tg[order]
    m_ = tgo // T
    t_ = tgo % T
    eo = order
    slot_src[m_, t_, j, p] = src[eo]
    slot_dstrel[m_, t_, j, p] = lane_of[dst[eo]].astype(np.float16)
    return dict(core_of=core_of, tile_of=tile_of, lane_of=lane_of,
                new2old=new2old, NT=NT, slot_src=slot_src,
                slot_dstrel=slot_dstrel, slot_idx=(m_, t_, j, p), eo=eo,
                s1=slot_ew1, s2=slot_ew2)


def _fill_ew(cfg, plan, ew1, ew2):
    m_, t_, j, p = plan["slot_idx"]
    eo = plan["eo"]
    plan["s1"][m_, t_, j, p] = np.asarray(ew1, np.float32).ravel()[eo]
    plan["s2"][m_, t_, j, p] = np.asarray(ew2, np.float32).ravel()[eo]


def _msg_table(cfg, plan, table16):
    """Per-edge replicated src rows: [M, 128, T*F*NT], col (t*F+ch)*NT+j."""
    M, T, NT = cfg.M, cfg.TILES, plan["NT"]
    d = table16.shape[1]
    g = table16[plan["slot_src"]]              # [M, T, NT, 128, d]
    g = np.ascontiguousarray(g.transpose(0, 3, 1, 4, 2))  # [M,128,T,d,NT]
    return g.reshape(M, 128, T * d * NT)


def _aux_table(cfg, plan):
    """[M, 128, T*3*NT], cols per tile: dst|ew1|ew2 each [NT]."""
    M, T, NT = cfg.M, cfg.TILES, plan["NT"]
    a = np.stack([plan["slot_dstrel"], plan["s1"], plan["s2"]], axis=2)
    # [M, T, 3, NT, 128] -> [M, 128, T, 3, NT]
    a = np.ascontiguousarray(a.transpose(0, 4, 1, 2, 3))
    return a.reshape(M, 128, T * 3 * NT)


def _own_table(cfg, plan, table16):
    """[M, d, T*PN]: column t*PN+n = row of own node (0 for pad lanes)."""
    M = cfg.M
    d = table16.shape[1]
    n2o = plan["new2old"]                      # [M, SLOTS]
    rows = table16[np.maximum(n2o, 0)]         # [M, SLOTS, d]
    rows[n2o < 0] = 0
    return np.ascontiguousarray(rows.transpose(0, 2, 1))


def _iota_const(cfg, NT):
    io = np.zeros((128, cfg.PN * NT), np.float16)
    io[:] = (np.arange(cfg.PN)[:, None] * np.ones((1, NT))).reshape(1, -1)
    return io


# ------------------------------------------------------------- device build

def _build_block(cfg, d_in, d_out, NT, out_f32):
    f16 = mybir.dt.float16
    f32 = mybir.dt.float32
    T, PN, GRP, SB = cfg.TILES, cfg.PN, cfg.GRP, cfg.SB
    GROUPS = T // GRP
    odt = f32 if out_f32 else f16

    nc = bacc.Bacc("TRN2", target_bir_lowering=False, debug=False,
                   num_devices=cfg.M)
    msg = nc.dram_tensor("msg", [128, T * d_in * NT], f16,
                         kind="ExternalInput")
    aux = nc.dram_tensor("aux", [128, T * 3 * NT], f16, kind="ExternalInput")
    own = nc.dram_tensor("own", [d_in, T * PN], f16, kind="ExternalInput")
    wts = nc.dram_tensor("wts", [d_in, 3 * d_out], f16, kind="ExternalInput")
    iot = nc.dram_tensor("iot", [128, PN * NT], f16, kind="ExternalInput")
    bia = nc.dram_tensor("bia", [d_out, 1], f32, kind="ExternalInput")
    out = nc.dram_tensor("out", [d_out, T * PN], odt, kind="ExternalOutput")

    MW = d_in * NT      # msg cols per tile
    AW = 3 * NT         # aux cols per tile

    with tile.TileContext(nc) as tc:
        with (
            tc.tile_pool(name="const", bufs=1) as cpool,
            tc.tile_pool(name="io", bufs=2) as iop,
            tc.tile_pool(name="sb", bufs=3) as pool,
            tc.tile_pool(name="ps", bufs=3, space="PSUM") as psum,
            tc.tile_pool(name="ps2", bufs=2, space="PSUM") as psum2,
        ):
            wts_t = cpool.tile([d_in, 3 * d_out], f16, tag="wts")
            nc.sync.dma_start(out=wts_t[:], in_=wts[:, :])
            iot_t = cpool.tile([128, PN * NT], f16, tag="iot")
            nc.sync.dma_start(out=iot_t[:], in_=iot[:, :])
            bia_t = cpool.tile([d_out, 1], f32, tag="bia")
            nc.sync.dma_start(out=bia_t[:], in_=bia[:, :])

            def tail(prev):
                # second-stage matmuls + bias for an earlier quad of tiles;
                # emitted after the next quad's aggregation so PE doesn't
                # block on the ACT evacuation
                a12s, own_g, out_g, base = prev
                ypp = psum2.tile([d_out, SB * PN], f32, tag="y", space="PSUM")
                for tt in range(SB):
                    nc.tensor.matmul(
                        out=ypp[:, tt * PN:(tt + 1) * PN],
                        lhsT=wts_t[:, 0:d_out],
                        rhs=a12s[:, tt * 2 * PN: tt * 2 * PN + PN],
                        start=True, stop=False)
                    nc.tensor.matmul(
                        out=ypp[:, tt * PN:(tt + 1) * PN],
                        lhsT=wts_t[:, d_out:2 * d_out],
                        rhs=a12s[:, tt * 2 * PN + PN:(tt + 1) * 2 * PN],
                        start=False, stop=False)
                    nc.tensor.matmul(
                        out=ypp[:, tt * PN:(tt + 1) * PN],
                        lhsT=wts_t[:, 2 * d_out:3 * d_out],
                        rhs=own_g[:, (base + tt) * PN:(base + tt + 1) * PN],
                        start=False, stop=True)
                nc.scalar.activation(
                    out=out_g[:, base * PN:(base + SB) * PN], in_=ypp[:],
                    func=mybir.ActivationFunctionType.Identity,
                    bias=bia_t[:, 0:1])

            prev = None
            for g in range(GROUPS):
                msg_g = iop.tile([128, GRP * MW], f16, tag="msg")
                W = GRP * MW
                me = [getattr(nc, e) for e in cfg.MSG_ENGINES]
                cw = W // len(me)
                for k, eng in enumerate(me):
                    lo = k * cw
                    hi = (k + 1) * cw if k < len(me) - 1 else W
                    eng.dma_start(
                        out=msg_g[:, lo:hi],
                        in_=msg[:, g * W + lo: g * W + hi])
                aux_g = iop.tile([128, GRP * AW], f16, tag="aux")
                nc.gpsimd.dma_start(
                    out=aux_g[:], in_=aux[:, g * GRP * AW:(g + 1) * GRP * AW])
                own_g = iop.tile([d_in, GRP * PN], f16, tag="own")
                nc.gpsimd.dma_start(
                    out=own_g[:], in_=own[:, g * GRP * PN:(g + 1) * GRP * PN])
                out_g = iop.tile([d_out, GRP * PN], odt, tag="out")

                for q in range(GRP // SB):
                    base = q * SB
                    # every k-th S-build runs on the Pool engine to unload DVE
                    ksm = cfg.POOL_S_MOD
                    veng = nc.gpsimd if ksm and (
                        g * (GRP // SB) + q) % ksm == ksm - 1 else nc.vector
                    # S matrices for SB tiles in 3 DVE ops (fp16, 2x mode)
                    s_q = pool.tile([128, SB * PN * NT], f16, tag="s")
                    s12q = pool.tile([128, SB * 2 * PN * NT], f16, tag="s12")
                    sqr = s_q[:].rearrange("p (t n j) -> p t n j", t=SB, j=NT)
                    s12r = s12q[:].rearrange("p (t n j) -> p t n j",
                                             t=SB, j=NT)
                    aux4 = aux_g[:, base * AW:(base + SB) * AW].rearrange(
                        "p (t k j) -> p t k j", t=SB, j=NT)
                    iob = iot_t[:].rearrange(
                        "p (n j) -> p n j", j=NT).unsqueeze(1).to_broadcast(
                        [128, SB, PN, NT])
                    veng.tensor_tensor(
                        out=sqr,
                        in0=aux4[:, :, 0].unsqueeze(2).to_broadcast(
                            [128, SB, PN, NT]),
                        in1=iob, op=mybir.AluOpType.is_equal)
                    veng.tensor_tensor(
                        out=s12r[:, :, 0:PN], in0=sqr,
                        in1=aux4[:, :, 1].unsqueeze(2).to_broadcast(
                            [128, SB, PN, NT]),
                        op=mybir.AluOpType.mult)
                    veng.tensor_tensor(
                        out=s12r[:, :, PN:2 * PN], in0=sqr,
                        in1=aux4[:, :, 2].unsqueeze(2).to_broadcast(
                            [128, SB, PN, NT]),
                        op=mybir.AluOpType.mult)

                    # aggregation: one matmul per (tile, subtile), both
                    # branches via rhs = [S1|S2]; SB tiles share one PSUM bank
                    a12p = psum.tile([d_in, SB * 2 * PN], f32, tag="a12",
                                     space="PSUM")
                    for tt in range(SB):
                        msg3 = msg_g[:, (base + tt) * MW:
                                     (base + tt + 1) * MW].rearrange(
                            "p (c j) -> p c j", j=NT)
                        for jj in range(NT):
                            nc.tensor.matmul(
                                out=a12p[:, tt * 2 * PN:(tt + 1) * 2 * PN],
                                lhsT=msg3[:, :, jj],
                                rhs=s12r[:, tt, :, jj],
                                start=(jj == 0), stop=(jj == NT - 1))
                    if prev is not None:
                        tail(prev)
                    a12s = pool.tile([d_in, SB * 2 * PN], f16, tag="a12s")
                    nc.scalar.activation(
                        out=a12s[:], in_=a12p[:],
                        func=mybir.ActivationFunctionType.Copy)
                    prev = (a12s, own_g, out_g, base)
                    if q == GRP // SB - 1:
                        tail(prev)      # flush before out_g is stored
                        prev = None

                getattr(nc, cfg.OUT_ENGINE).dma_start(
                    out=out[:, g * GRP * PN:(g + 1) * GRP * PN],
                    in_=out_g[:])

    nc.compile()
    return nc


_BUILD_CACHE = {}


def _get_block(cfg, d_in, d_out, NT, out_f32):
    key = (cfg.N, cfg.E, cfg.PN, cfg.GRP, cfg.TILES, d_in, d_out, NT, out_f32)
    if key not in _BUILD_CACHE:
        _BUILD_CACHE[key] = _build_block(cfg, d_in, d_out, NT, out_f32)
    return _BUILD_CACHE[key]


def _run_block(cfg, ncb, msg, aux, own, wts, iot, bia):
    in_maps = []
    for c in range(cfg.M):
        in_maps.append({
            "msg": msg[c], "aux": aux[c], "own": own[c],
            "wts": wts, "iot": iot, "bia": bia,
        })
    res = bass_utils.run_bass_kernel_spmd(
        ncb, in_maps, core_ids=list(range(cfg.M)))
    return np.stack([r["out"] for r in res.results])   # [M, d_out, SLOTS]


def _prep_plan(cfg, src, dst, ew1, ew2):
    plan = _plan(cfg, src, dst)
    _fill_ew(cfg, plan, ew1, ew2)
    plan["aux"] = _aux_table(cfg, plan)
    plan["iot"] = _iota_const(cfg, plan["NT"])
    return plan


def _wts16(w1, w2, wl, b1, b2, bl):
    w = np.ascontiguousarray(np.concatenate(
        [w1, w2, wl], axis=1), np.float32).astype(np.float16)
    b = np.ascontiguousarray(
        (np.asarray(bl) + np.asarray(b1) + np.asarray(b2))
        .reshape(-1, 1), np.float32)
    return w, b


def _prep_block(cfg, plan, table16, d_out, wts, bia, own_t=None):
    """(nc, in_maps-style arrays) for one inception block."""
    d_in = table16.shape[1]
    msg = _msg_table(cfg, plan, table16)
    own = own_t if own_t is not None else _own_table(cfg, plan, table16)
    nc = _get_block(cfg, d_in, d_out, plan["NT"], out_f32=False)
    return nc, (msg, plan["aux"], own, wts, plan["iot"], bia)


def _unperm(cfg, plan, rT, dtype):
    """[M, d, SLOTS] device output -> [N, d] in original node order."""
    full = np.zeros((cfg.N, rT.shape[1]), dtype)
    for m in range(cfg.M):
        n2o = plan["new2old"][m]
        real = n2o >= 0
        full[n2o[real]] = rT[m][:, real].T
    return full


def _kernel_cfg(cfg, features, ew1, ew2, src, dst,
                ln1_w, ln1_b, c11_w, c11_b, c12_w, c12_b,
                ln2_w, ln2_b, c21_w, c21_b, c22_w, c22_b):
    plan = _prep_plan(cfg, src, dst, ew1, ew2)
    feat16 = np.ascontiguousarray(features, np.float32).astype(np.float16)
    wts1, bia1 = _wts16(c11_w, c12_w, ln1_w, c11_b, c12_b, ln1_b)
    nc1, args1 = _prep_block(cfg, plan, feat16, cfg.EMB, wts1, bia1)
    xT = _run_block(cfg, nc1, *args1)
    x16 = _unperm(cfg, plan, xT, np.float16)

    wts2, bia2 = _wts16(c21_w, c22_w, ln2_w, c21_b, c22_b, ln2_b)
    nc2, args2 = _prep_block(cfg, plan, x16, cfg.OUT, wts2, bia2,
                             own_t=np.ascontiguousarray(xT))
    yT = _run_block(cfg, nc2, *args2)
    return _unperm(cfg, plan, yT, np.float32).astype(np.float32)


def kernel(features, ew1, ew2, src, dst,
           ln1_w, ln1_b, c11_w, c11_b, c12_w, c12_b,
           ln2_w, ln2_b, c21_w, c21_b, c22_w, c22_b):
    return _kernel_cfg(FULL, features, ew1, ew2, src, dst,
                       ln1_w, ln1_b, c11_w, c11_b, c12_w, c12_b,
                       ln2_w, ln2_b, c21_w, c21_b, c22_w, c22_b)
